# revision 11
# baseline (speedup 1.0000x reference)
"""Grouped-experts GEMM (MoE ragged dot) on 8 TRN2 NeuronCores.

Reference semantics (jax.lax.ragged_dot):
    for each expert e with contiguous token group [start_e, end_e):
        out[start_e:end_e] = input[start_e:end_e] @ weight[e]
    rows beyond sum(tokens_per_expert) are zero.

Sharding: tensor-parallel over out_features. Every core sees all tokens
(identical expert boundaries -> identical SPMD program on all 8 cores,
as required by run_bass_kernel_spmd's single-NEFF shard_map) and
computes a disjoint 512-wide slice of the 4096 output columns; the
"gather" is a host-side concatenate, no collectives needed.

Compute: bf16 operands with fp32 PSUM accumulation. bf16 streams the PE
at 1 row/cycle (~216ns per 128x128x512 matmul) and halves operand bytes
vs fp32, making the kernel compute-bound (~64MB DMA vs ~223us of
matmul per core). Measured: ~246us HW exec, rel err ~2.4e-3 vs the
fp32 reference (x ~ N(0,1), w ~ 0.02*N(0,1), K=2048).

Layout: X is pre-transposed/tiled on host to [chunk, p(k), kt, t]
(2 M-tiles per 1MB chunk) so every DMA is contiguous per partition;
W per core is [expert, p(k), kt, n]. K=2048 is accumulated over 16
matmuls into one PSUM bank per (m-tile, 512-wide n-block). X and W loads ride the sync HWDGE ring; output stores ride the
scalar ring so they never block the input-load FIFO.
"""

import sys

import numpy as np

sys.path.insert(0, "/opt/trn_rl_repo")

import ml_dtypes

NUM_TOKENS = 8192
IN_FEATURES = 2048
OUT_FEATURES = 4096
GROUPS = 8
N_CORES = 8

P = 128
KT = IN_FEATURES // P  # 16 K-tiles of 128
N_CORE = OUT_FEATURES // N_CORES  # 512 output cols per core

_BUILD_CACHE: dict = {}


def _build_program(units: tuple[int, ...]):
    """Build the single SPMD program, specialized to the per-expert
    padded M-tile counts `units` (identical on every core)."""
    import concourse.mybir as mybir
    import concourse.tile as tile
    from concourse import bacc

    f32 = mybir.dt.float32
    bf16 = mybir.dt.bfloat16
    U = sum(units)
    assert U % 2 == 0
    C = U // 2  # X chunks of 2 M-tiles (1MB each)

    nc = bacc.Bacc(None, target_bir_lowering=False)
    x_p = nc.declare_dram_parameter("x", [C, P, KT, 2 * P], bf16, isOutput=False)
    w_p = nc.declare_dram_parameter("w", [GROUPS, P, KT, N_CORE], bf16, isOutput=False)
    o_p = nc.declare_dram_parameter("out", [U * P, N_CORE], f32, isOutput=True)

    with tile.TileContext(nc) as tc:
        with (
            tc.tile_pool(name="xp", bufs=4) as xpool,
            tc.tile_pool(name="wp", bufs=3) as wpool,
            tc.tile_pool(name="op", bufs=4) as opool,
            tc.tile_pool(name="ps", bufs=4, space="PSUM") as pspool,
        ):
            owner = []
            for e in range(GROUPS):
                owner += [e] * units[e]
            w_cur = (None, None)
            x_cur = (None, None)
            for m in range(U):
                e = owner[m]
                if w_cur[0] != e:
                    w_t = wpool.tile([P, KT, N_CORE], bf16, tag="w")
                    # first W via SWDGE (gpsimd): Pool has no table-load
                    # preamble, so its DMA issues ~6us before SP/ACT can
                    weng = nc.gpsimd if w_cur[0] is None else nc.sync
                    weng.dma_start(out=w_t[:], in_=w_p[e])
                    w_cur = (e, w_t)
                w_t = w_cur[1]
                c, half = divmod(m, 2)
                if x_cur[0] != c:
                    x_t = xpool.tile([P, KT, 2 * P], bf16, tag="x")
                    xeng = nc.gpsimd if x_cur[0] is None else nc.sync
                    xeng.dma_start(out=x_t[:], in_=x_p[c])
                    x_cur = (c, x_t)
                x_t = x_cur[1]
                ps = pspool.tile([P, N_CORE], f32, tag="ps")
                for k in range(KT):
                    nc.tensor.matmul(
                        ps[:],
                        x_t[:, k, half * P : (half + 1) * P],
                        w_t[:, k, :],
                        start=(k == 0),
                        stop=(k == KT - 1),
                    )
                o_t = opool.tile([P, N_CORE], f32, tag="o")
                nc.vector.tensor_copy(o_t[:], ps[:])
                nc.scalar.dma_start(out=o_p[m * P : (m + 1) * P, :], in_=o_t[:])
    nc.compile()
    return nc


def _get_program(units: tuple[int, ...]):
    if units not in _BUILD_CACHE:
        _BUILD_CACHE[units] = _build_program(units)
    return _BUILD_CACHE[units]


def _segments(tokens_per_expert: np.ndarray, total: int):
    """Per-expert sizes with ragged_dot clipping semantics."""
    sizes = []
    start = 0
    for e in range(GROUPS):
        s = int(max(0, tokens_per_expert[e]))
        s = min(s, total - start)
        sizes.append(s)
        start += s
    return sizes


def kernel(input, weight, tokens_per_expert, _trace=False, _trace_kwargs=None):
    from concourse.bass_utils import run_bass_kernel_spmd

    x = np.asarray(input, dtype=np.float32)
    w = np.asarray(weight, dtype=np.float32)
    tpe = np.asarray(tokens_per_expert, dtype=np.int64)
    T, K = x.shape
    G, K2, N = w.shape
    assert (T, K, G, K2, N) == (NUM_TOKENS, IN_FEATURES, GROUPS, IN_FEATURES, OUT_FEATURES)

    sizes = _segments(tpe, T)
    units = [-(-s // P) for s in sizes]  # ceil(s/128)
    if sum(units) % 2:
        for e in range(GROUPS):  # make U even for 2-M-tile X chunks
            if units[e] > 0 or e == GROUPS - 1:
                units[e] += 1
                break
    units = tuple(units)
    U = sum(units)
    out = np.zeros((T, N), dtype=np.float32)
    if U == 0:
        return out

    # Padded token layout: each expert's rows start at a 128-multiple.
    mstarts = np.concatenate([[0], np.cumsum(units)])[:GROUPS]
    Xp = np.zeros((U * P, K), dtype=ml_dtypes.bfloat16)
    start = 0
    for e in range(GROUPS):
        s = sizes[e]
        if s:
            Xp[mstarts[e] * P : mstarts[e] * P + s] = x[start : start + s]
        start += s

    # x_dram[c, p, kt, t] = Xp[c*256 + t, kt*128 + p]
    C = U // 2
    x_dram = np.ascontiguousarray(Xp.reshape(C, 2 * P, KT, P).transpose(0, 3, 2, 1))
    wb = w.astype(ml_dtypes.bfloat16)
    # w_dram[c][e, p, kt, n] = w[e, kt*128 + p, c*512 + n]
    w_drams = [
        np.ascontiguousarray(
            wb[:, :, c * N_CORE : (c + 1) * N_CORE]
            .reshape(G, KT, P, N_CORE)
            .transpose(0, 2, 1, 3)
        )
        for c in range(N_CORES)
    ]

    nc = _get_program(units)
    in_maps = [{"x": x_dram, "w": w_drams[c]} for c in range(N_CORES)]
    kw = dict(_trace_kwargs or {})
    res = run_bass_kernel_spmd(nc, in_maps, list(range(N_CORES)), trace=_trace, **kw)
    full = np.concatenate(
        [res.results[c]["out"] for c in range(N_CORES)], axis=1
    )  # [U*128, 4096]

    start = 0
    for e in range(GROUPS):
        s = sizes[e]
        if s:
            out[start : start + s] = full[mstarts[e] * P : mstarts[e] * P + s]
        start += s
    if _trace:
        return out, res
    return out


# revision 13
# speedup vs baseline: 1.0805x; 1.0805x over previous
"""Grouped-experts GEMM (MoE ragged dot) on 8 TRN2 NeuronCores.

Reference semantics (jax.lax.ragged_dot):
    for each expert e with contiguous token group [start_e, end_e):
        out[start_e:end_e] = input[start_e:end_e] @ weight[e]
    rows beyond sum(tokens_per_expert) are zero.

Sharding: tensor-parallel over out_features. Every core sees all tokens
(identical expert boundaries -> identical SPMD program on all 8 cores,
as required by run_bass_kernel_spmd's single-NEFF shard_map) and
computes a disjoint 512-wide slice of the 4096 output columns; the
"gather" is a host-side concatenate, no collectives needed.

Compute: bf16 operands with fp32 PSUM accumulation. bf16 streams the PE
at 1 row/cycle (~216ns per 128x128x512 matmul) and halves operand bytes
vs fp32, making the kernel compute-bound (~64MB DMA vs ~223us of
matmul per core). Measured: ~246us HW exec, rel err ~2.4e-3 vs the
fp32 reference (x ~ N(0,1), w ~ 0.02*N(0,1), K=2048).

Layout: X is pre-transposed/tiled on host to [chunk, p(k), kt, t]
(2 M-tiles per 1MB chunk) so every DMA is contiguous per partition;
W per core is [expert, p(k), kt, n]. K=2048 is accumulated over 16
matmuls into one PSUM bank per (m-tile, 512-wide n-block). X and W loads ride the sync HWDGE ring; output stores ride the
scalar ring so they never block the input-load FIFO.
"""

import sys

import numpy as np

sys.path.insert(0, "/opt/trn_rl_repo")

import ml_dtypes

NUM_TOKENS = 8192
IN_FEATURES = 2048
OUT_FEATURES = 4096
GROUPS = 8
N_CORES = 8

P = 128
KT = IN_FEATURES // P  # 16 K-tiles of 128
N_CORE = OUT_FEATURES // N_CORES  # 512 output cols per core

_BUILD_CACHE: dict = {}


def _build_program(units: tuple[int, ...]):
    """Build the single SPMD program, specialized to the per-expert
    padded M-tile counts `units` (identical on every core)."""
    import concourse.mybir as mybir
    import concourse.tile as tile
    from concourse import bacc

    f32 = mybir.dt.float32
    bf16 = mybir.dt.bfloat16
    U = sum(units)
    assert U % 2 == 0
    C = U // 2  # X chunks of 2 M-tiles (1MB each)

    nc = bacc.Bacc(None, target_bir_lowering=False)
    x_p = nc.declare_dram_parameter("x", [C, P, KT, 2 * P], bf16, isOutput=False)
    w_p = nc.declare_dram_parameter("w", [GROUPS, P, KT, N_CORE], bf16, isOutput=False)
    o_p = nc.declare_dram_parameter("out", [U * P, N_CORE], f32, isOutput=True)

    with tile.TileContext(nc) as tc:
        with (
            tc.tile_pool(name="xp", bufs=4) as xpool,
            tc.tile_pool(name="wp", bufs=3) as wpool,
            tc.tile_pool(name="wh", bufs=2) as whpool,
            tc.tile_pool(name="op", bufs=4) as opool,
            tc.tile_pool(name="ps", bufs=4, space="PSUM") as pspool,
        ):
            owner = []
            for e in range(GROUPS):
                owner += [e] * units[e]

            # First expert's W as two K-halves, emission-ordered
            # half0 -> first X chunk -> half1, so the first matmul waits
            # on 2MB of loads instead of 3MB (all on the sync ring).
            e0 = owner[0]
            wh0 = whpool.tile([P, KT // 2, N_CORE], bf16, tag="wh")
            nc.sync.dma_start(out=wh0[:], in_=w_p[e0, :, : KT // 2])
            w_halves = [wh0, None]

            w_cur = (e0, None)  # None -> use w_halves
            x_cur = (None, None)
            for m in range(U):
                e = owner[m]
                if w_cur[0] != e:
                    w_t = wpool.tile([P, KT, N_CORE], bf16, tag="w")
                    nc.sync.dma_start(out=w_t[:], in_=w_p[e])
                    w_cur = (e, w_t)
                w_t = w_cur[1]
                c, half = divmod(m, 2)
                if x_cur[0] != c:
                    x_t = xpool.tile([P, KT, 2 * P], bf16, tag="x")
                    nc.sync.dma_start(out=x_t[:], in_=x_p[c])
                    x_cur = (c, x_t)
                    if m == 0:
                        wh1 = whpool.tile([P, KT // 2, N_CORE], bf16, tag="wh")
                        nc.sync.dma_start(out=wh1[:], in_=w_p[e0, :, KT // 2 :])
                        w_halves[1] = wh1
                x_t = x_cur[1]
                ps = pspool.tile([P, N_CORE], f32, tag="ps")
                for k in range(KT):
                    if w_t is None:
                        w_ap = w_halves[k // (KT // 2)][:, k % (KT // 2), :]
                    else:
                        w_ap = w_t[:, k, :]
                    nc.tensor.matmul(
                        ps[:],
                        x_t[:, k, half * P : (half + 1) * P],
                        w_ap,
                        start=(k == 0),
                        stop=(k == KT - 1),
                    )
                o_t = opool.tile([P, N_CORE], f32, tag="o")
                nc.vector.tensor_copy(o_t[:], ps[:])
                nc.scalar.dma_start(out=o_p[m * P : (m + 1) * P, :], in_=o_t[:])
    nc.compile()
    return nc


def _get_program(units: tuple[int, ...]):
    if units not in _BUILD_CACHE:
        _BUILD_CACHE[units] = _build_program(units)
    return _BUILD_CACHE[units]


def _segments(tokens_per_expert: np.ndarray, total: int):
    """Per-expert sizes with ragged_dot clipping semantics."""
    sizes = []
    start = 0
    for e in range(GROUPS):
        s = int(max(0, tokens_per_expert[e]))
        s = min(s, total - start)
        sizes.append(s)
        start += s
    return sizes


def kernel(input, weight, tokens_per_expert, _trace=False, _trace_kwargs=None):
    from concourse.bass_utils import run_bass_kernel_spmd

    x = np.asarray(input, dtype=np.float32)
    w = np.asarray(weight, dtype=np.float32)
    tpe = np.asarray(tokens_per_expert, dtype=np.int64)
    T, K = x.shape
    G, K2, N = w.shape
    assert (T, K, G, K2, N) == (NUM_TOKENS, IN_FEATURES, GROUPS, IN_FEATURES, OUT_FEATURES)

    sizes = _segments(tpe, T)
    units = [-(-s // P) for s in sizes]  # ceil(s/128)
    if sum(units) % 2:
        for e in range(GROUPS):  # make U even for 2-M-tile X chunks
            if units[e] > 0 or e == GROUPS - 1:
                units[e] += 1
                break
    units = tuple(units)
    U = sum(units)
    out = np.zeros((T, N), dtype=np.float32)
    if U == 0:
        return out

    # Padded token layout: each expert's rows start at a 128-multiple.
    mstarts = np.concatenate([[0], np.cumsum(units)])[:GROUPS]
    Xp = np.zeros((U * P, K), dtype=ml_dtypes.bfloat16)
    start = 0
    for e in range(GROUPS):
        s = sizes[e]
        if s:
            Xp[mstarts[e] * P : mstarts[e] * P + s] = x[start : start + s]
        start += s

    # x_dram[c, p, kt, t] = Xp[c*256 + t, kt*128 + p]
    C = U // 2
    x_dram = np.ascontiguousarray(Xp.reshape(C, 2 * P, KT, P).transpose(0, 3, 2, 1))
    wb = w.astype(ml_dtypes.bfloat16)
    # w_dram[c][e, p, kt, n] = w[e, kt*128 + p, c*512 + n]
    w_drams = [
        np.ascontiguousarray(
            wb[:, :, c * N_CORE : (c + 1) * N_CORE]
            .reshape(G, KT, P, N_CORE)
            .transpose(0, 2, 1, 3)
        )
        for c in range(N_CORES)
    ]

    nc = _get_program(units)
    in_maps = [{"x": x_dram, "w": w_drams[c]} for c in range(N_CORES)]
    kw = dict(_trace_kwargs or {})
    res = run_bass_kernel_spmd(nc, in_maps, list(range(N_CORES)), trace=_trace, **kw)
    full = np.concatenate(
        [res.results[c]["out"] for c in range(N_CORES)], axis=1
    )  # [U*128, 4096]

    start = 0
    for e in range(GROUPS):
        s = sizes[e]
        if s:
            out[start : start + s] = full[mstarts[e] * P : mstarts[e] * P + s]
        start += s
    if _trace:
        return out, res
    return out


# revision 14
# speedup vs baseline: 1.0856x; 1.0048x over previous
"""Grouped-experts GEMM (MoE ragged dot) on 8 TRN2 NeuronCores.

Reference semantics (jax.lax.ragged_dot):
    for each expert e with contiguous token group [start_e, end_e):
        out[start_e:end_e] = input[start_e:end_e] @ weight[e]
    rows beyond sum(tokens_per_expert) are zero.

Sharding: tensor-parallel over out_features. Every core sees all tokens
(identical expert boundaries -> identical SPMD program on all 8 cores,
as required by run_bass_kernel_spmd's single-NEFF shard_map) and
computes a disjoint 512-wide slice of the 4096 output columns; the
"gather" is a host-side concatenate, no collectives needed.

Compute: bf16 operands with fp32 PSUM accumulation. bf16 streams the PE
at 1 row/cycle (~216ns per 128x128x512 matmul) and halves operand bytes
vs fp32, making the kernel compute-bound (~64MB DMA vs ~223us of
matmul per core). Measured: ~246us HW exec, rel err ~2.4e-3 vs the
fp32 reference (x ~ N(0,1), w ~ 0.02*N(0,1), K=2048).

Layout: X is pre-transposed/tiled on host to [chunk, p(k), kt, t]
(2 M-tiles per 1MB chunk) so every DMA is contiguous per partition;
W per core is [expert, p(k), kt, n]. K=2048 is accumulated over 16
matmuls into one PSUM bank per (m-tile, 512-wide n-block). X and W loads ride the sync HWDGE ring; output stores ride the
scalar ring so they never block the input-load FIFO.
"""

import sys

import numpy as np

sys.path.insert(0, "/opt/trn_rl_repo")

import ml_dtypes

NUM_TOKENS = 8192
IN_FEATURES = 2048
OUT_FEATURES = 4096
GROUPS = 8
N_CORES = 8

P = 128
KT = IN_FEATURES // P  # 16 K-tiles of 128
N_CORE = OUT_FEATURES // N_CORES  # 512 output cols per core

_BUILD_CACHE: dict = {}


def _build_program(units: tuple[int, ...]):
    """Build the single SPMD program, specialized to the per-expert
    padded M-tile counts `units` (identical on every core)."""
    import concourse.mybir as mybir
    import concourse.tile as tile
    from concourse import bacc

    f32 = mybir.dt.float32
    bf16 = mybir.dt.bfloat16
    U = sum(units)
    assert U % 2 == 0
    C = U // 2  # X chunks of 2 M-tiles (1MB each)

    nc = bacc.Bacc(None, target_bir_lowering=False)
    x_p = nc.declare_dram_parameter("x", [C, P, KT, 2 * P], bf16, isOutput=False)
    w_p = nc.declare_dram_parameter("w", [GROUPS, P, KT, N_CORE], bf16, isOutput=False)
    o_p = nc.declare_dram_parameter("out", [U * P, N_CORE], f32, isOutput=True)

    with tile.TileContext(nc) as tc:
        with (
            tc.tile_pool(name="xp", bufs=4) as xpool,
            tc.tile_pool(name="wp", bufs=3) as wpool,
            tc.tile_pool(name="wh", bufs=2) as whpool,
            tc.tile_pool(name="xh", bufs=2) as xhpool,
            tc.tile_pool(name="op", bufs=4) as opool,
            tc.tile_pool(name="ps", bufs=4, space="PSUM") as pspool,
        ):
            owner = []
            for e in range(GROUPS):
                owner += [e] * units[e]

            # First expert's W as two K-halves, emission-ordered
            # half0 -> first X chunk -> half1, so the first matmul waits
            # on 2MB of loads instead of 3MB (all on the sync ring).
            e0 = owner[0]
            wh0 = whpool.tile([P, KT // 2, N_CORE], bf16, tag="wh")
            nc.sync.dma_start(out=wh0[:], in_=w_p[e0, :, : KT // 2])
            w_halves = [wh0, None]
            # first X chunk likewise as two K-halves: the first matmul's
            # critical path is then W-h0 + X-h0 = 1.5MB
            x_halves = []
            for hh in range(2):
                x_h = xhpool.tile([P, KT // 2, 2 * P], bf16, tag="xh")
                nc.sync.dma_start(
                    out=x_h[:], in_=x_p[0, :, hh * (KT // 2) : (hh + 1) * (KT // 2)]
                )
                if hh == 0:
                    wh1 = whpool.tile([P, KT // 2, N_CORE], bf16, tag="wh")
                    nc.sync.dma_start(out=wh1[:], in_=w_p[e0, :, KT // 2 :])
                    w_halves[1] = wh1
                x_halves.append(x_h)

            w_cur = (e0, None)  # None -> use w_halves
            x_cur = (0, None)  # None -> use x_halves
            for m in range(U):
                e = owner[m]
                if w_cur[0] != e:
                    w_t = wpool.tile([P, KT, N_CORE], bf16, tag="w")
                    nc.sync.dma_start(out=w_t[:], in_=w_p[e])
                    w_cur = (e, w_t)
                w_t = w_cur[1]
                c, half = divmod(m, 2)
                if x_cur[0] != c:
                    x_t = xpool.tile([P, KT, 2 * P], bf16, tag="x")
                    nc.sync.dma_start(out=x_t[:], in_=x_p[c])
                    x_cur = (c, x_t)
                x_t = x_cur[1]
                ps = pspool.tile([P, N_CORE], f32, tag="ps")
                for k in range(KT):
                    if w_t is None:
                        w_ap = w_halves[k // (KT // 2)][:, k % (KT // 2), :]
                    else:
                        w_ap = w_t[:, k, :]
                    if x_t is None:
                        x_ap = x_halves[k // (KT // 2)][
                            :, k % (KT // 2), half * P : (half + 1) * P
                        ]
                    else:
                        x_ap = x_t[:, k, half * P : (half + 1) * P]
                    nc.tensor.matmul(
                        ps[:],
                        x_ap,
                        w_ap,
                        start=(k == 0),
                        stop=(k == KT - 1),
                    )
                o_t = opool.tile([P, N_CORE], f32, tag="o")
                nc.vector.tensor_copy(o_t[:], ps[:])
                nc.scalar.dma_start(out=o_p[m * P : (m + 1) * P, :], in_=o_t[:])
    nc.compile()
    return nc


def _get_program(units: tuple[int, ...]):
    if units not in _BUILD_CACHE:
        _BUILD_CACHE[units] = _build_program(units)
    return _BUILD_CACHE[units]


def _segments(tokens_per_expert: np.ndarray, total: int):
    """Per-expert sizes with ragged_dot clipping semantics."""
    sizes = []
    start = 0
    for e in range(GROUPS):
        s = int(max(0, tokens_per_expert[e]))
        s = min(s, total - start)
        sizes.append(s)
        start += s
    return sizes


def kernel(input, weight, tokens_per_expert, _trace=False, _trace_kwargs=None):
    from concourse.bass_utils import run_bass_kernel_spmd

    x = np.asarray(input, dtype=np.float32)
    w = np.asarray(weight, dtype=np.float32)
    tpe = np.asarray(tokens_per_expert, dtype=np.int64)
    T, K = x.shape
    G, K2, N = w.shape
    assert (T, K, G, K2, N) == (NUM_TOKENS, IN_FEATURES, GROUPS, IN_FEATURES, OUT_FEATURES)

    sizes = _segments(tpe, T)
    units = [-(-s // P) for s in sizes]  # ceil(s/128)
    if sum(units) % 2:
        for e in range(GROUPS):  # make U even for 2-M-tile X chunks
            if units[e] > 0 or e == GROUPS - 1:
                units[e] += 1
                break
    units = tuple(units)
    U = sum(units)
    out = np.zeros((T, N), dtype=np.float32)
    if U == 0:
        return out

    # Padded token layout: each expert's rows start at a 128-multiple.
    mstarts = np.concatenate([[0], np.cumsum(units)])[:GROUPS]
    Xp = np.zeros((U * P, K), dtype=ml_dtypes.bfloat16)
    start = 0
    for e in range(GROUPS):
        s = sizes[e]
        if s:
            Xp[mstarts[e] * P : mstarts[e] * P + s] = x[start : start + s]
        start += s

    # x_dram[c, p, kt, t] = Xp[c*256 + t, kt*128 + p]
    C = U // 2
    x_dram = np.ascontiguousarray(Xp.reshape(C, 2 * P, KT, P).transpose(0, 3, 2, 1))
    wb = w.astype(ml_dtypes.bfloat16)
    # w_dram[c][e, p, kt, n] = w[e, kt*128 + p, c*512 + n]
    w_drams = [
        np.ascontiguousarray(
            wb[:, :, c * N_CORE : (c + 1) * N_CORE]
            .reshape(G, KT, P, N_CORE)
            .transpose(0, 2, 1, 3)
        )
        for c in range(N_CORES)
    ]

    nc = _get_program(units)
    in_maps = [{"x": x_dram, "w": w_drams[c]} for c in range(N_CORES)]
    kw = dict(_trace_kwargs or {})
    res = run_bass_kernel_spmd(nc, in_maps, list(range(N_CORES)), trace=_trace, **kw)
    full = np.concatenate(
        [res.results[c]["out"] for c in range(N_CORES)], axis=1
    )  # [U*128, 4096]

    start = 0
    for e in range(GROUPS):
        s = sizes[e]
        if s:
            out[start : start + s] = full[mstarts[e] * P : mstarts[e] * P + s]
        start += s
    if _trace:
        return out, res
    return out
